# revision 3
# baseline (speedup 1.0000x reference)
"""Trainium2 Bass kernel for nn_DataWindowLoss: mean(|box5x5(x) - box5x5(y)|).

Math: the 5x5 uniform box filter (padding=4) is linear and separable, so
    box(x) - box(y) = box(x - y) = A @ (x - y) @ A^T   (per image)
where A is the [516, 512] banded matrix with A[o, r] = 1 for o-4 <= r <= o.
Band entries are 1.0; the division by 25 happens on the host.

The kernel is HBM-read-bound (16.8 MB fp32/core). Inputs are DMA'd in the
c2 layout, rearrange (k p c) w -> p k c w (p=128, c=2, k=2): partition p
holds row pairs {256k + 2p, 256k + 2p + 1}, so every HBM descriptor is a
4KB contiguous read — measured 2-4 us faster than the plain (k p) layout's
2KB descriptors in both the 1-core and 8-core-contended regimes.

d = x - y is computed on the VectorEngine (fp16 subtract, one op per
k-half), which keeps the TensorEngine affordable despite the c2 layout's
wider bands: pass1 contracts per (k, c) tile with the slope-2 band
b_c[p, o] = +1 iff 2p+c <= o <= 2p+c+4 (6 matmuls, 1040 PE columns per
w-block; single-source, so the 2x band width costs less than the old
two-source slope-1 form). ACT drains PSUM -> SBUF fp16.

pass2 for images 0..6 (flipped): H^T[o, c] = sum_w VT[w, o] * B[w, c]
with vt slices as stationary weights and the 132-wide band streaming
(528 PE columns per o-block); the LAST image uses the band-stationary
form instead, whose chunks need only vt w-blocks m-1..m and therefore
interleave per-m with pass1 — the critical path after the final DMA
stays ~1.5 us. The last image's y is loaded as two per-k-half DMAs
(each half is a contiguous 256-row block, keeping 4KB descriptors) so
the tail subtract fires per arriving half. DVE abs-sum-reduces each
pass2 PSUM block into one fp32 accumulator column.

Schedule: pass2 of image i-1 interleaves between pass1 m-tiles of image
i (software pipeline on the in-order PE queue, 2-buf PSUM pools);
subtracts for image i+1 are emitted at the end of image block i so the
in-order DVE queue never stalls on an in-flight DMA ahead of ready
reduces. All input DMAs are issued up front so the single SWDGE queue
never drains.

Sharding: pure data parallel - 8 images per core on 8 NeuronCores; each
core emits a [128, 40] fp32 partial-sum tile (5 abs-sum columns x 8
images; the column decomposition differs between the two pass2 forms but
the host just sums everything); the host reduces and normalizes.
"""

import sys

sys.path.insert(0, "/opt/trn_rl_repo")

import numpy as np

import concourse.mybir as mybir
import concourse.tile as tile
from concourse import bacc
from concourse.bass_utils import run_bass_kernel_spmd

N_CORES = 8
IMG_PER_CORE = 8
P = 128          # partitions
HW = 512         # image height/width
KT = 4           # w-blocks per image
OUT = 516        # output spatial size (512 + 2*4 - 5 + 1)
F16 = mybir.dt.float16
F32 = mybir.dt.float32


def _make_band_consts(nc, pool):
    """bandP [128, 132]: band[p, j] = 1 iff p <= j <= p+4 (pass2).
    bandL [128, 128]: corner[p, q] = 1 iff p - q >= 124 (last-image pass2).
    """
    bandP = pool.tile([P, 132], F16)
    bandL = pool.tile([P, 128], F16)
    nc.gpsimd.memset(bandP, 1.0)
    nc.gpsimd.affine_select(
        out=bandP, in_=bandP, compare_op=mybir.AluOpType.is_ge, fill=0.0,
        base=0, pattern=[[1, 132]], channel_multiplier=-1)
    nc.gpsimd.affine_select(
        out=bandP, in_=bandP, compare_op=mybir.AluOpType.is_ge, fill=0.0,
        base=4, pattern=[[-1, 132]], channel_multiplier=1)
    nc.gpsimd.memset(bandL, 1.0)
    nc.gpsimd.affine_select(
        out=bandL, in_=bandL, compare_op=mybir.AluOpType.is_ge, fill=0.0,
        base=-124, pattern=[[-1, 128]], channel_multiplier=1)
    return bandP, bandL


def _make_c2_consts(nc, pool):
    """Slope-2 positive bands for c2-layout pass1.

    b{0,1} [128, 260]: b_c[p, o] = 1 iff 2p+c <= o <= 2p+c+4
    """
    out = []
    for c in (0, 1):
        t = pool.tile([P, 260], F16, name=f"c2band{c}")
        nc.gpsimd.memset(t, 1.0)
        nc.gpsimd.affine_select(
            out=t, in_=t, compare_op=mybir.AluOpType.is_ge, fill=0.0,
            base=-c, pattern=[[1, 260]], channel_multiplier=-2)
        nc.gpsimd.affine_select(
            out=t, in_=t, compare_op=mybir.AluOpType.is_ge, fill=0.0,
            base=c + 4, pattern=[[-1, 260]], channel_multiplier=2)
        out.append(t)
    return out


def build_module():
    nc = bacc.Bacc()
    x_dram = nc.dram_tensor("x", [IMG_PER_CORE, HW, HW], F32,
                            kind="ExternalInput")
    y_dram = nc.dram_tensor("y", [IMG_PER_CORE, HW, HW], F32,
                            kind="ExternalInput")
    out_dram = nc.dram_tensor("partials", [P, IMG_PER_CORE * 5], F32,
                              kind="ExternalOutput")

    with tile.TileContext(nc) as tc:
        with (
            tc.tile_pool(name="consts", bufs=1) as consts_pool,
            tc.tile_pool(name="xin", bufs=8) as xpool,
            tc.tile_pool(name="yin", bufs=8) as ypool,
            tc.tile_pool(name="dd", bufs=3) as ddpool,
            tc.tile_pool(name="y7p", bufs=1) as y7pool,
            tc.tile_pool(name="vt", bufs=4) as vtpool,
            tc.tile_pool(name="accp", bufs=1) as accpool,
            tc.tile_pool(name="vtps", bufs=2, space="PSUM") as vt_ps_pool,
            tc.tile_pool(name="hps", bufs=2, space="PSUM") as h_ps_pool,
        ):
            # All input DMAs issued up front; image 0's x-load traces before
            # the const-building so HBM traffic starts before the memsets.
            xs, ys, ds = [], [], []
            for i in range(IMG_PER_CORE):
                x_sb = xpool.tile([P, 2, 2, HW], F16, name="x_sb")
                nc.gpsimd.dma_start(
                    out=x_sb,
                    in_=x_dram[i].rearrange("(k p c) w -> p k c w",
                                            p=P, c=2))
                xs.append(x_sb)
                if i == 0:
                    bandP, bandL = _make_band_consts(nc, consts_pool)
                    b0, b1 = _make_c2_consts(nc, consts_pool)
                if i < IMG_PER_CORE - 1:
                    y_sb = ypool.tile([P, 2, 2, HW], F16, name="y_sb")
                    nc.gpsimd.dma_start(
                        out=y_sb,
                        in_=y_dram[i].rearrange("(k p c) w -> p k c w",
                                                p=P, c=2))
                    ys.append(y_sb)
            # last image's y as two per-k-half DMAs (contiguous 256-row
            # blocks keep the 4KB descriptors) for a short tail subtract
            y7k = []
            for k in range(2):
                t = y7pool.tile([P, 2, HW], F16, name=f"y7k{k}")
                nc.gpsimd.dma_start(
                    out=t,
                    in_=y_dram[IMG_PER_CORE - 1, 256 * k:256 * (k + 1), :]
                    .rearrange("(p c) w -> p c w", p=P))
                y7k.append(t)

            # 5 abs-sum columns per image, fp32
            acc = accpool.tile([P, IMG_PER_CORE * 5], F32)
            nc.vector.memset(acc, 0.0)

            def emit_sub(i):
                dk = []
                for k in range(2):
                    d = ddpool.tile([P, 2, HW], F16, name=f"d{k}")
                    yin = ys[i][:, k] if i < IMG_PER_CORE - 1 else y7k[k]
                    nc.vector.tensor_tensor(
                        out=d, in0=xs[i][:, k], in1=yin,
                        op=mybir.AluOpType.subtract)
                    dk.append(d)
                ds.append(dk)

            def p1c2(di, vt, m):
                wb = slice(128 * m, 128 * (m + 1))
                vt_ps = vt_ps_pool.tile([P, OUT], F32)
                # PSUM window protocol: bank0 [0:512) opened by the first
                # matmul, closed by the last bank0 touch; bank1 [512:516)
                # has its own open/close pair.
                nc.tensor.matmul(
                    vt_ps[:, 0:260], lhsT=di[0][:, 0, wb],
                    rhs=b0[:, 0:260], start=True, stop=False)
                nc.tensor.matmul(
                    vt_ps[:, 0:260], lhsT=di[0][:, 1, wb],
                    rhs=b1[:, 0:260], start=False, stop=False)
                nc.tensor.matmul(
                    vt_ps[:, 256:512], lhsT=di[1][:, 0, wb],
                    rhs=b0[:, 0:256], start=False, stop=False)
                nc.tensor.matmul(
                    vt_ps[:, 512:516], lhsT=di[1][:, 0, wb],
                    rhs=b0[:, 256:260], start=True, stop=False)
                nc.tensor.matmul(
                    vt_ps[:, 256:512], lhsT=di[1][:, 1, wb],
                    rhs=b1[:, 0:256], start=False, stop=True)
                nc.tensor.matmul(
                    vt_ps[:, 512:516], lhsT=di[1][:, 1, wb],
                    rhs=b1[:, 256:260], start=False, stop=True)
                nc.scalar.copy(out=vt[:, m, :], in_=vt_ps)

            def p2flip(vt_prev, ob, jimg):
                o0 = 128 * ob
                opn = 128 if ob < 4 else 4
                osl = slice(o0, o0 + opn)
                psl = slice(0, opn)
                h_ps = h_ps_pool.tile([P, OUT], F32)
                for m in range(KT):
                    c0 = 128 * m
                    lhsT = vt_prev[:, m, osl]
                    if m < 3:
                        nc.tensor.matmul(
                            h_ps[psl, c0:c0 + 132], lhsT=lhsT,
                            rhs=bandP[:, 0:132], start=(m == 0), stop=False)
                    else:
                        nc.tensor.matmul(
                            h_ps[psl, 384:512], lhsT=lhsT,
                            rhs=bandP[:, 0:128], start=False, stop=True)
                        nc.tensor.matmul(
                            h_ps[psl, 512:516], lhsT=lhsT,
                            rhs=bandP[:, 128:132], start=True, stop=True)
                nc.vector.tensor_reduce(
                    out=acc[psl, jimg * 5 + ob:jimg * 5 + ob + 1],
                    in_=h_ps[psl, :],
                    axis=mybir.AxisListType.X,
                    op=mybir.AluOpType.add,
                    apply_absolute_value=True,
                )

            def p2old(vt7, m, jimg):
                h_ps = h_ps_pool.tile([P, OUT], F32)
                if m == 0:
                    psl = slice(0, P)
                    parts = [(bandP[:, 0:128], 0)]
                elif m <= 3:
                    psl = slice(0, P)
                    parts = [(bandP[:, 0:128], m), (bandL, m - 1)]
                else:
                    psl = slice(0, 4)
                    parts = [(bandL[:, 0:4], 3)]
                for j, (lhsT, wsrc) in enumerate(parts):
                    first, last = j == 0, j == len(parts) - 1
                    nc.tensor.matmul(
                        h_ps[psl, 0:512], lhsT=lhsT,
                        rhs=vt7[:, wsrc, 0:512], start=first, stop=last)
                    nc.tensor.matmul(
                        h_ps[psl, 512:516], lhsT=lhsT,
                        rhs=vt7[:, wsrc, 512:516], start=first, stop=last)
                nc.vector.tensor_reduce(
                    out=acc[psl, jimg * 5 + m:jimg * 5 + m + 1],
                    in_=h_ps[psl, :],
                    axis=mybir.AxisListType.X,
                    op=mybir.AluOpType.add,
                    apply_absolute_value=True,
                )

            emit_sub(0)
            prev = None  # (img_idx, vt tile) pending flipped pass2
            for i in range(IMG_PER_CORE):
                vt = vtpool.tile([P, KT, OUT], F16)
                last_img = i == IMG_PER_CORE - 1
                for m in range(KT):
                    p1c2(ds[i], vt, m)
                    if prev is not None:
                        p2flip(prev[1], m, prev[0])
                    if last_img and m >= 1:
                        p2old(vt, m - 1, i)
                if prev is not None:
                    p2flip(prev[1], 4, prev[0])
                if last_img:
                    p2old(vt, 3, i)
                    p2old(vt, 4, i)
                if i + 1 < IMG_PER_CORE:
                    emit_sub(i + 1)
                prev = (i, vt)

            nc.sync.dma_start(out=out_dram[:], in_=acc)

    nc.finalize()
    return nc


_NC_CACHE = None


def kernel(x: np.ndarray, y: np.ndarray) -> np.ndarray:
    global _NC_CACHE
    if _NC_CACHE is None:
        _NC_CACHE = build_module()
    nc = _NC_CACHE

    x = np.ascontiguousarray(
        np.asarray(x, dtype=np.float32).reshape(64, HW, HW))
    y = np.ascontiguousarray(
        np.asarray(y, dtype=np.float32).reshape(64, HW, HW))

    in_maps = [
        {
            "x": x[IMG_PER_CORE * c:IMG_PER_CORE * (c + 1)],
            "y": y[IMG_PER_CORE * c:IMG_PER_CORE * (c + 1)],
        }
        for c in range(N_CORES)
    ]
    res = run_bass_kernel_spmd(nc, in_maps, core_ids=list(range(N_CORES)))
    total = np.float64(0.0)
    for r in res.results:
        total += r["partials"].astype(np.float64).sum()
    mean = total / (25.0 * 64 * OUT * OUT)
    return np.float32(mean)
